# revision 6
# baseline (speedup 1.0000x reference)
"""Causal self-attention Trainium2 Bass kernel (bf16 matmul pipeline).

Problem: B=4, T=2048, E=1024, H=16 heads, D=64. fp32 in/out.
Sharding: 8 cores = 4 batches x 2 head-groups (8 heads each).
Per-core pipeline (all matmuls bf16 operands, fp32 PSUM accumulate):
  QKV projections -> Q^T,K^T [c,t] + V [t,c] (ones-augmented) on SBUF
  per head-pair, per 2-k-tile group:
    S^T = K Q^T  (k on partitions, q on free dim)
    exp via scalar ACT (scale=1/8 fused) -> probs bf16
    causal band tiles: column-restricted + bandmask multiply on DVE
  PV transposed: stationary = probs [k,128q], moving = V65 [k,65]
    -> po_t [q, 4*(64 out | rowsum)] accumulated in PSUM
  normalize: DVE reciprocal of rowsums [q,4] + per-partition
    tensor_scalar_mul on gpsimd -> att_t [q, c] bf16
  PE transpose (identity matmul) -> attn [c, t] bf16
  output projection -> partial [T, E] fp32, summed pairwise on host.
"""

import sys

sys.path.insert(0, "/opt/trn_rl_repo")

import numpy as np
import ml_dtypes

import concourse.bass as bass
import concourse.tile as tile
from concourse import bacc, mybir
from concourse.bass import ds, ts
from concourse.bass_utils import run_bass_kernel_spmd

F32 = mybir.dt.float32
BF16 = mybir.dt.bfloat16
AF = mybir.ActivationFunctionType
NPBF16 = ml_dtypes.bfloat16

P = 128
E = 1024
HEADS_PER_CORE = 8
C = HEADS_PER_CORE * 64  # 512 head-dims per core
EO = E // P  # 8
CO = C // P  # 4
TQ = 512  # q tile width
N_CORES = 8


def build_nc(T=2048, reps=1):
    NT = T // TQ  # q tiles of 512
    nc = bacc.Bacc("TRN2", target_bir_lowering=False, debug=False)

    xt_d = nc.dram_tensor("xt", [P, EO, T], BF16, kind="ExternalInput").ap()
    wq_d = nc.dram_tensor("wq", [P, EO, C], BF16, kind="ExternalInput").ap()
    wk_d = nc.dram_tensor("wk", [P, EO, C], BF16, kind="ExternalInput").ap()
    wv_d = nc.dram_tensor("wv", [P, EO, C], BF16, kind="ExternalInput").ap()
    wo_d = nc.dram_tensor("wo", [P, CO, E], BF16, kind="ExternalInput").ap()
    bq_d = nc.dram_tensor("bq", [P, CO], F32, kind="ExternalInput").ap()
    bk_d = nc.dram_tensor("bk", [P, CO], F32, kind="ExternalInput").ap()
    bvb_d = nc.dram_tensor("bvb", [P, C], F32, kind="ExternalInput").ap()
    # [128, 256]: cols 0-127 zero, cols 128-255 lower-band triangle (kk<=cc)
    mk_d = nc.dram_tensor("bandmask", [P, 256], BF16, kind="ExternalInput").ap()
    on_d = nc.dram_tensor("onesc", [P, 128], BF16, kind="ExternalInput").ap()
    id_d = nc.dram_tensor("ident", [P, 128], BF16, kind="ExternalInput").ap()
    out_d = nc.dram_tensor("out", [T, E], F32, kind="ExternalOutput").ap()

    with tile.TileContext(nc) as tc, \
         tc.tile_pool(name="psc", bufs=2, space="PSUM") as psc, \
         tc.tile_pool(name="ppo", bufs=3, space="PSUM") as ppo, \
         tc.tile_pool(name="ptr", bufs=1, space="PSUM") as ptr:
        # PSUM banks: psc 2x2 + ppo 3x1 + ptr 1 = 8
        for _ in range(reps):
            with tc.tile_pool(name="pers", bufs=1) as pers:
                QT = pers.tile([P, CO, T], BF16, tag="qt")
                KT = pers.tile([P, CO, T], BF16, tag="kt")
                V65 = pers.tile([P, T // P, 520], BF16, tag="v65")
                ident = pers.tile([P, 128], BF16, tag="id")

                # ---------------- phase 1: QKV projections ----------------
                with (
                    tc.tile_pool(name="ph1w", bufs=1) as ph1w,
                    tc.tile_pool(name="ph1x", bufs=1) as ph1x,
                ):
                    wq_sb = ph1w.tile([P, EO, C], BF16, tag="wq")
                    wk_sb = ph1w.tile([P, EO, C], BF16, tag="wk")
                    wv_sb = ph1w.tile([P, EO, C], BF16, tag="wv")
                    bq_sb = ph1w.tile([P, CO], F32, tag="bq")
                    bk_sb = ph1w.tile([P, CO], F32, tag="bk")
                    bvb_sb = ph1w.tile([P, C], F32, tag="bvb")
                    XT = ph1x.tile([P, EO, T], BF16, tag="xt")
                    # critical-path first: wq co=0 chunk + x first eo slices
                    nc.sync.dma_start(wq_sb[:, :, ts(0, P)], wq_d[:, :, ts(0, P)])
                    for eo in range(EO):
                        nc.sync.dma_start(XT[:, eo, ts(0, TQ)], xt_d[:, eo, ts(0, TQ)])
                    nc.sync.dma_start(bq_sb[:], bq_d)
                    for co_d in range(1, CO):
                        nc.sync.dma_start(
                            wq_sb[:, :, ts(co_d, P)], wq_d[:, :, ts(co_d, P)]
                        )
                    for ti_d in range(1, NT):
                        nc.sync.dma_start(
                            XT[:, :, ts(ti_d, TQ)], xt_d[:, :, ts(ti_d, TQ)]
                        )
                    nc.sync.dma_start(bk_sb[:], bk_d)
                    for co_d in range(CO):
                        nc.sync.dma_start(
                            wk_sb[:, :, ts(co_d, P)], wk_d[:, :, ts(co_d, P)]
                        )
                    nc.sync.dma_start(bvb_sb[:], bvb_d)
                    nc.sync.dma_start(wv_sb[:], wv_d)
                    nc.sync.dma_start(ident[:], id_d)
                    # ones column of each V-block
                    v_ones = V65.rearrange("p k (h w) -> p k h w", w=65)[:, :, :, 64]
                    nc.sync.dma_start(
                        v_ones,
                        on_d[:, : (T // P) * 8].rearrange("p (a b) -> p a b", b=8),
                    )

                    for ti in range(NT):
                        xt_t = XT[:, :, ts(ti, TQ)]
                        # Q^T and K^T tiles [c=128, t=512]
                        for dst, w_sb, b_sb in (
                            (QT, wq_sb, bq_sb),
                            (KT, wk_sb, bk_sb),
                        ):
                            for co in range(CO):
                                pt = psc.tile([P, 1024], F32, tag="sc", name="pt_p")
                                for eo in range(EO):
                                    nc.tensor.matmul(
                                        pt[:, :TQ],
                                        w_sb[:, eo, ts(co, P)],
                                        xt_t[:, eo, :],
                                        start=(eo == 0),
                                        stop=(eo == EO - 1),
                                    )
                                nc.vector.tensor_scalar_add(
                                    dst[:, co, ts(ti, TQ)],
                                    pt[:, :TQ],
                                    b_sb[:, co : co + 1],
                                )
                        # V tiles [t=128, c=512] -> V65 blocks + bias
                        for tsub in range(TQ // P):
                            kt_i = ti * (TQ // P) + tsub
                            pt = psc.tile([P, 1024], F32, tag="sc", name="pt_v")
                            for eo in range(EO):
                                nc.tensor.matmul(
                                    pt[:, :C],
                                    xt_t[:, eo, ts(tsub, P)],
                                    wv_sb[:, eo, :],
                                    start=(eo == 0),
                                    stop=(eo == EO - 1),
                                )
                            v_dst = V65[:, kt_i, :].rearrange(
                                "p (h w) -> p h w", w=65
                            )[:, :, 0:64]
                            nc.vector.tensor_add(
                                v_dst,
                                pt[:, :C].rearrange("p (h w) -> p h w", w=64),
                                bvb_sb.rearrange("p (h w) -> p h w", w=64),
                            )

                # -------- phase 2: attention + output projection --------
                with (
                    tc.tile_pool(name="ph2", bufs=1) as ph2,
                    tc.tile_pool(name="ppr", bufs=6) as ppr,
                    tc.tile_pool(name="pat", bufs=2) as pat,
                    tc.tile_pool(name="psm", bufs=4) as psm,
                    tc.tile_pool(name="pob", bufs=2) as pob,
                ):
                    wo_sb = ph2.tile([P, CO, E], BF16, tag="wo")
                    mk_sb = ph2.tile([P, 256], BF16, tag="mk")
                    attn = ph2.tile([P, CO, T], BF16, tag="attn")
                    nc.sync.dma_start(wo_sb[:], wo_d)
                    nc.sync.dma_start(mk_sb[:], mk_d)

                    def oproj_gen(qi):
                        for tsub in range(TQ // P):
                            tt = qi * (TQ // P) + tsub
                            for e2 in range(E // TQ):
                                pt = psc.tile(
                                    [P, 1024], F32, tag="sc", name="pt_op"
                                )
                                for co in range(CO):
                                    nc.tensor.matmul(
                                        pt[:, :TQ],
                                        attn[:, co, ds(tt * P, P)],
                                        wo_sb[:, co, ts(e2, TQ)],
                                        start=(co == 0),
                                        stop=(co == CO - 1),
                                    )
                                ob = pob.tile([P, TQ], F32, tag="ob", name="ob")
                                nc.vector.tensor_copy(ob[:], pt[:, :TQ])
                                nc.sync.dma_start(
                                    out_d[ds(tt * P, P), ts(e2, TQ)], ob[:]
                                )
                                yield

                    def drain(it, n):
                        if it is None:
                            return None
                        for _ in range(n):
                            if next(it, "end") == "end":
                                return None
                        return it

                    op_it = None

                    for qi in range(NT):
                        nkt = 4 * qi + 4  # causal k-tiles of 128
                        ng = nkt // 2  # groups of 2 k-tiles
                        for hp in range(4):  # head pairs
                            if hp == 0 and qi > 0:
                                op_it = oproj_gen(qi - 1)
                            po = {}
                            for s in (0, 1):
                                po[s] = ppo.tile(
                                    [P, 4, 65], F32, tag="pot", name=f"po{s}"
                                )
                            for g in range(ng):
                                band = g >= ng - 2
                                # per k-tile effective q-column start
                                c0 = {}
                                for u in (0, 1):
                                    rr = 2 * (g - (ng - 2)) + u if band else 0
                                    c0[u] = min(128 * rr, 256) if band else 0
                                pr = {}
                                for s in (0, 1):  # head 2hp+s
                                    ps_t = psc.tile(
                                        [P, 1024], F32, tag="sc", name="ps"
                                    )
                                    lo, hi = 64 * s, 64 * s + 64
                                    for u in (0, 1):
                                        kt_i = 2 * g + u
                                        nc.tensor.matmul(
                                            ps_t[:, ds(u * TQ + c0[u], TQ - c0[u])],
                                            KT[lo:hi, hp, ds(kt_i * P, P)],
                                            QT[lo:hi, hp, ds(qi * TQ + c0[u], TQ - c0[u])],
                                            start=True,
                                            stop=True,
                                        )
                                    pr[s] = ppr.tile(
                                        [P, 1024], BF16, tag="pr", name="pr"
                                    )
                                    if not band:
                                        nc.scalar.activation(
                                            pr[s][:], ps_t[:], AF.Exp, scale=0.125
                                        )
                                    else:
                                        for u in (0, 1):
                                            sl = ds(u * TQ + c0[u], TQ - c0[u])
                                            nc.scalar.activation(
                                                pr[s][:, sl],
                                                ps_t[:, sl],
                                                AF.Exp,
                                                scale=0.125,
                                            )
                                            rr = 2 * (g - (ng - 2)) + u
                                            w = 128 * rr + 128 - c0[u]
                                            msl = ds(u * TQ + c0[u], w)
                                            nc.vector.tensor_mul(
                                                pr[s][:, msl],
                                                pr[s][:, msl],
                                                mk_sb[:, ds(256 - w, w)],
                                            )
                                # PV transposed: out [q, 65] per q-sub-tile.
                                # start/stop are per PSUM *bank*: one long
                                # group per po[s]; untouched slot bytes are
                                # zeroed on first write after start.
                                for s in (0, 1):
                                    h = 2 * hp + s
                                    for u in (0, 1):
                                        kt_i = 2 * g + u
                                        rr = 2 * (g - (ng - 2)) + u if band else 0
                                        for qs in range(rr, 4):
                                            nc.tensor.matmul(
                                                po[s][:, qs, :],
                                                pr[s][:, ds(u * TQ + qs * P, P)],
                                                V65[:, kt_i, ds(65 * h, 65)],
                                                start=(kt_i == 0 and qs == 0),
                                                stop=(kt_i == nkt - 1 and qs == 3),
                                            )
                                op_it = drain(op_it, 3)
                            # normalize (gpsimd) + PE-transpose to [c, t]
                            att_t = pat.tile([P, 512], BF16, tag="att")
                            for s in (0, 1):
                                rcpv = psm.tile([P, 4], F32, tag="rcp")
                                with nc.allow_low_precision(
                                    reason="recip fine for softmax"
                                ):
                                    nc.vector.reciprocal(
                                        rcpv[:], po[s][:, :, 64]
                                    )
                                for qs in range(4):
                                    nc.vector.tensor_scalar_mul(
                                        att_t[:, ds(qs * P + s * 64, 64)],
                                        po[s][:, qs, 0:64],
                                        rcpv[:, qs : qs + 1],
                                    )
                            ptr_t = ptr.tile([P, 4, 128], BF16, tag="tr")
                            for qs in range(4):
                                nc.tensor.transpose(
                                    ptr_t[:, qs, :],
                                    att_t[:, ds(qs * P, P)],
                                    ident[:],
                                )
                                nc.vector.tensor_copy(
                                    attn[:, hp, ds(qi * TQ + qs * P, P)],
                                    ptr_t[:, qs, :],
                                )
                    drain(op_it, 100)
                    for _ in oproj_gen(NT - 1):
                        pass
    nc.compile()
    return nc


def build_null_nc():
    """Tiny kernel used to measure per-dispatch overhead."""
    nc = bacc.Bacc("TRN2", target_bir_lowering=False, debug=False)
    z_d = nc.dram_tensor("z", [1, 128], F32, kind="ExternalInput").ap()
    o_d = nc.dram_tensor("o", [1, 128], F32, kind="ExternalOutput").ap()
    with tile.TileContext(nc) as tc:
        with tc.tile_pool(name="sb", bufs=1) as sb:
            t = sb.tile([1, 128], F32, tag="t")
            nc.sync.dma_start(t[:], z_d)
            nc.sync.dma_start(o_d, t[:])
    nc.compile()
    return nc


# ---------------------------------------------------------------------------
# host side


def _bandmask():
    kk = np.arange(P)[:, None]
    cc = np.arange(128)[None, :]
    m = np.zeros((P, 256), dtype=np.float32)
    m[:, 128:] = (kk <= cc).astype(np.float32)
    return m.astype(NPBF16)


def _per_core_inputs(x, Wq, bq, Wk, bk, Wv, bv, Wo, T):
    """Build the 8 per-core input dicts (host-side slicing/layout)."""
    bandmask = _bandmask()
    ident = np.eye(P, dtype=np.float32).astype(NPBF16)
    onesc = np.ones((P, 128), dtype=np.float32).astype(NPBF16)
    in_maps = []
    for c in range(N_CORES):
        b, hg = c // 2, c % 2
        hs = slice(C * hg, C * (hg + 1))

        def to_pet(a, n_outer):  # [E_like, F] -> [P, n_outer, F] bf16
            return np.ascontiguousarray(
                a.reshape(n_outer, P, a.shape[-1])
                .transpose(1, 0, 2)
                .astype(NPBF16)
            )

        xt = to_pet(np.ascontiguousarray(x[b].T.astype(np.float32)), EO)
        wq = to_pet(np.ascontiguousarray(Wq[hs].T.astype(np.float32)), EO)
        wk = to_pet(np.ascontiguousarray(Wk[hs].T.astype(np.float32)), EO)
        wv = to_pet(np.ascontiguousarray(Wv[hs].T.astype(np.float32)), EO)
        wo = to_pet(np.ascontiguousarray(Wo[:, hs].T.astype(np.float32)), CO)
        in_maps.append(
            {
                "xt": xt,
                "wq": wq,
                "wk": wk,
                "wv": wv,
                "wo": wo,
                "bq": np.ascontiguousarray(
                    bq[hs].astype(np.float32).reshape(CO, P).T
                ),
                "bk": np.ascontiguousarray(
                    bk[hs].astype(np.float32).reshape(CO, P).T
                ),
                "bvb": np.ascontiguousarray(
                    np.broadcast_to(bv[hs].astype(np.float32), (P, C))
                ),
                "bandmask": bandmask,
                "onesc": onesc,
                "ident": ident,
            }
        )
    return in_maps


_NC_CACHE = {}


def _get_nc(T, reps=1):
    key = (T, reps)
    if key not in _NC_CACHE:
        _NC_CACHE[key] = build_nc(T, reps)
    return _NC_CACHE[key]


def kernel(x, Wq, bq, Wk, bk, Wv, bv, Wo, bo):
    x = np.asarray(x, dtype=np.float32)
    B, T, _ = x.shape
    nc = _get_nc(T)
    in_maps = _per_core_inputs(
        x,
        np.asarray(Wq),
        np.asarray(bq),
        np.asarray(Wk),
        np.asarray(bk),
        np.asarray(Wv),
        np.asarray(bv),
        np.asarray(Wo),
        T,
    )
    res = run_bass_kernel_spmd(nc, in_maps, core_ids=list(range(N_CORES)))
    bo32 = np.asarray(bo, dtype=np.float32)
    out = np.empty((B, T, E), dtype=np.float32)
    for b in range(B):
        out[b] = res.results[2 * b]["out"] + res.results[2 * b + 1]["out"] + bo32
    return out
